# revision 2
# baseline (speedup 1.0000x reference)
"""Per-domain batch normalization (BaseDomainBatchNorm) on 8 Trainium2 NeuronCores.

Math (reference):
    cnt[j]   = #{n : d[n] == j}            (clamped to >= 1)
    mean[j]  = sum_{d[n]==j} X[n] / cnt[j]
    var[j]   = sum_{d[n]==j} X[n]^2 / cnt[j] - mean[j]^2
    inv[j]   = rsqrt(var[j] + 1e-5)
    Y[n]     = X[n] * A[d[n]] + B[d[n]],  A = inv*gamma, B = beta - mean*A

Sharding: rows split 8192 per core; per-domain partial stats (sum/sumsq/cnt)
are AllReduce'd across the 8 cores; each core normalizes its own rows.

V2 design (vs the fp32 baseline):
  - X is loaded ONCE as bf16 via SWDGE cast-DMA in 8x 2MB transfers
    (fp32 HBM -> bf16 SBUF); no per-chunk f32->bf16 DVE casts.
  - stats per chunk: psum_s += onehot.T @ x_bf ; psum_q += onehot.T @ xsq
    (xsq alternates ACT Square / DVE mul to split engine load).
  - a tiny warm-up AllReduce issues at t~0 so the ~40us first-collective
    setup cost overlaps the X loads; the real stats AllReduce then runs
    at its marginal latency.
  - phase 2 per super-chunk (2 chunks, [128,2048] psum):
    4 gather matmuls produce [A(c0)|A(c1)|B(c0)|B(c1)] in one psum tile;
    one fp32->bf16 evacuation (ACT for most supers, DVE for the rest to
    balance); FMA on DVE in bf16 at 2x rate; result staged in bf16 and
    cast-stored (bf16 SBUF -> fp32 HBM) in 2MB transfers.
HBM traffic is the roofline minimum: read X once, write Y once.
"""

import numpy as np

N = 65536
C = 512
D = 16
NCORES = 8
SHARD = N // NCORES          # 8192 rows per core
P = 128                      # partitions
CHUNKS = SHARD // P          # 64 chunks of 128 rows
SUPERS = CHUNKS // 2         # 32 super-chunks
GROUPS = CHUNKS // 8         # 8 groups of 8 chunks (2MB DMA granularity)
EPS = 1e-5

_CACHE = {}


def _build_program():
    import concourse.bacc as bacc
    import concourse.tile as tile
    from concourse import mybir

    f32 = mybir.dt.float32
    bf16 = mybir.dt.bfloat16
    i32 = mybir.dt.int32
    Alu = mybir.AluOpType
    Act = mybir.ActivationFunctionType

    nc = bacc.Bacc("TRN2", target_bir_lowering=False, debug=False,
                   num_devices=NCORES)

    X_d = nc.dram_tensor("X", [SHARD, C], f32, kind="ExternalInput")
    d_d = nc.dram_tensor("d", [SHARD], i32, kind="ExternalInput")
    g_d = nc.dram_tensor("gamma", [D, C], f32, kind="ExternalInput")
    b_d = nc.dram_tensor("beta", [D, C], f32, kind="ExternalInput")
    Y_d = nc.dram_tensor("Y", [SHARD, C], f32, kind="ExternalOutput")

    ccw_in = nc.dram_tensor("ccw_in", [D, 1], f32)
    ccw_out = nc.dram_tensor("ccw_out", [D, 1], f32, addr_space="Shared")
    cc_in = nc.dram_tensor("cc_in", [D, 2 * C + 1], f32)
    cc_out = nc.dram_tensor("cc_out", [D, 2 * C + 1], f32, addr_space="Shared")

    # partition p owns rows [p*64, (p+1)*64)
    Xv = X_d.ap().rearrange("(p n) c -> p n c", p=P)   # [128, 64, 512]
    Yv = Y_d.ap().rearrange("(p n) c -> p n c", p=P)

    DB = 1024  # d-broadcast strip width

    with tile.TileContext(nc) as tc:
        with (
            tc.tile_pool(name="const", bufs=1) as cpool,
            tc.tile_pool(name="x", bufs=GROUPS) as xpool,
            tc.tile_pool(name="sq", bufs=3) as sqpool,
            tc.tile_pool(name="oh", bufs=1) as ohpool,
            tc.tile_pool(name="small", bufs=1) as spool,
            tc.tile_pool(name="scr", bufs=2) as scrpool,
            tc.tile_pool(name="dbc", bufs=2) as dbcpool,
            tc.tile_pool(name="ab", bufs=3) as abpool,
            tc.tile_pool(name="y", bufs=3) as ypool,
        ):
            # ---- warm-up collective: pays the first-collective setup cost
            # while the X loads stream in ----
            warm_t = spool.tile([D, 1], f32, tag="warm")
            nc.vector.memset(warm_t[:], 1.0)
            nc.scalar.dma_start(ccw_in[:], warm_t[:])
            nc.gpsimd.collective_compute(
                "AllReduce", Alu.add,
                replica_groups=[list(range(NCORES))],
                ins=[ccw_in[:]], outs=[ccw_out[:]])

            # ---- X loads: 8x 2MB cast-DMAs (fp32 HBM -> bf16 SBUF) ----
            xs = []
            for g in range(GROUPS):
                xt = xpool.tile([P, 8 * C], bf16)
                xs.append(xt)
                nc.gpsimd.dma_start(
                    xt[:].rearrange("p (n c) -> p n c", c=C),
                    Xv[:, 8 * g:8 * g + 8, :])

            # ---- constants ----
            iota_rep = cpool.tile([P, CHUNKS, D], bf16)
            nc.gpsimd.iota(iota_rep[:], pattern=[[0, CHUNKS], [1, D]], base=0,
                           channel_multiplier=0,
                           allow_small_or_imprecise_dtypes=True)
            iota_i = cpool.tile([2 * D, 1], i32)
            nc.gpsimd.iota(iota_i[:], pattern=[[0, 1]], base=0,
                           channel_multiplier=1)
            nc.vector.tensor_scalar(iota_i[:], iota_i[:], D - 1, None,
                                    Alu.bitwise_and)
            iota_col32 = cpool.tile([2 * D, 1], f32)
            nc.vector.tensor_copy(iota_col32[:], iota_i[:])
            ones_col = cpool.tile([P, 1], bf16)
            nc.vector.memset(ones_col[:], 1.0)

            # ---- d in chunk layout and one-hot [128, 64, 16] ----
            d_pn = cpool.tile([P, CHUNKS], i32)
            nc.sync.dma_start(d_pn[:], d_d.ap().rearrange("(p n) -> p n", p=P))
            d_f = cpool.tile([P, CHUNKS], bf16)
            nc.vector.tensor_copy(d_f[:], d_pn[:])
            onehot = ohpool.tile([P, CHUNKS, D], bf16)
            nc.vector.tensor_tensor(
                onehot[:], iota_rep[:],
                d_f[:].unsqueeze(-1).broadcast_to([P, CHUNKS, D]),
                Alu.is_equal)

            # ---- transposed one-hot [128, 8192]; rows 0:16 are the real
            # one-hot (table rows 16:128 are zero) ----
            onehotT = ohpool.tile([P, SHARD], bf16)
            for h in range(SHARD // DB):
                d_bc = dbcpool.tile([2 * D, DB], i32)
                src = d_d.ap()[h * DB:(h + 1) * DB]
                src = src.rearrange("(a n) -> a n", a=1).partition_broadcast(2 * D)
                nc.gpsimd.dma_start(d_bc[:], src)
                nc.vector.tensor_scalar(onehotT[0:2 * D, h * DB:(h + 1) * DB],
                                        d_bc[:], iota_col32[:], None,
                                        Alu.is_equal)
            # rows 32:128 only need *defined* values (their table rows are
            # zero): cheap SBUF->SBUF DMA copies
            for pb in range(2 * D, P, 2 * D):
                nc.gpsimd.dma_start(onehotT[pb:pb + 2 * D, :],
                                    onehotT[0:2 * D, :])

            # ---- AB2 table [128, 1024]: rows 0:16 = [A | B], rest zero ----
            AB2 = spool.tile([P, 2 * C], bf16, tag="AB2")
            for pb in range(2 * D, P, 2 * D):
                nc.vector.memset(AB2[pb:pb + 2 * D, :], 0.0)
            # rows 16:32 zeroed by SBUF->SBUF copy (32-partition write align)
            nc.gpsimd.dma_start(AB2[D:2 * D, :], AB2[2 * D:2 * D + D, :])

            # ---- gamma/beta early loads ----
            gam = spool.tile([D, C], f32, tag="gam")
            nc.scalar.dma_start(gam[:], g_d[:])
            bet = spool.tile([D, C], f32, tag="bet")
            nc.scalar.dma_start(bet[:], b_d[:])

            # ---- phase 1: per-core partial stats ----
            stats = spool.tile([D, 2 * C + 1], f32, tag="stats")
            with tc.tile_pool(name="ps1", bufs=1, space="PSUM") as ps1:
                psum_s = ps1.tile([D, C], f32)
                psum_q = ps1.tile([D, C], f32)
                psum_c = ps1.tile([D, 1], f32)
                for i in range(CHUNKS):
                    g, k = divmod(i, 8)
                    xsl = xs[g][:, k * C:(k + 1) * C]
                    xsq = sqpool.tile([P, C], bf16, tag="xsq")
                    if i % 2 == 0:
                        nc.scalar.activation(xsq[:], xsl, Act.Square)
                    else:
                        nc.vector.tensor_mul(xsq[:], xsl, xsl)
                    oh = onehot[:, i, :]
                    st, sp = (i == 0), (i == CHUNKS - 1)
                    nc.tensor.matmul(psum_s[:], oh, xsl,
                                     start=st, stop=sp)
                    nc.tensor.matmul(psum_q[:], oh, xsq[:],
                                     start=st, stop=sp)

                # counts: reduce one-hot over chunks, then one matmul
                rowcnt = spool.tile([P, D], f32, tag="rowcnt")
                nc.vector.tensor_reduce(
                    rowcnt[:], onehot[:].rearrange("p n d -> p d n"),
                    mybir.AxisListType.X, Alu.add)
                rowcnt_bf = spool.tile([P, D], bf16, tag="rowcnt_bf")
                nc.vector.tensor_copy(rowcnt_bf[:], rowcnt[:])
                nc.tensor.matmul(psum_c[:], rowcnt_bf[:], ones_col[:],
                                 start=True, stop=True)

                nc.vector.tensor_copy(stats[:, 0:C], psum_s[:])
                nc.vector.tensor_copy(stats[:, C:2 * C], psum_q[:])
                nc.vector.tensor_copy(stats[:, 2 * C:2 * C + 1], psum_c[:])

                # keep the PE HAM clock-gate warm across the all-reduce stall
                warm = ps1.tile([P, C], f32)
                for _ in range(14):
                    nc.tensor.matmul(warm[:], onehotT[:, 0:P],
                                     onehotT[:, 0:C],
                                     start=True, stop=True,
                                     skip_group_check=True)

            # ---- all-reduce partial stats across the 8 cores ----
            nc.sync.dma_start(cc_in[:], stats[:])
            nc.gpsimd.collective_compute(
                "AllReduce", Alu.add,
                replica_groups=[list(range(NCORES))],
                ins=[cc_in[:]], outs=[cc_out[:]])
            red = spool.tile([D, 2 * C + 1], f32, tag="red")
            nc.sync.dma_start(red[:], cc_out[:])

            # ---- finalize: A = inv*gamma, B = beta - mean*A ----
            cntc = spool.tile([D, 1], f32, tag="cntc")
            nc.vector.tensor_scalar_max(cntc[:], red[:, 2 * C:2 * C + 1], 1.0)
            rinv = spool.tile([D, 1], f32, tag="rinv")
            nc.vector.reciprocal(rinv[:], cntc[:])
            mean = spool.tile([D, C], f32, tag="mean")
            nc.vector.tensor_scalar_mul(mean[:], red[:, 0:C], rinv[:])
            var = spool.tile([D, C], f32, tag="var")
            nc.vector.tensor_scalar_mul(var[:], red[:, C:2 * C], rinv[:])
            negm2 = scrpool.tile([D, C], f32, tag="scr")
            nc.vector.scalar_tensor_tensor(negm2[:], mean[:], -1.0, mean[:],
                                           Alu.mult, Alu.mult)
            nc.vector.tensor_add(var[:], var[:], negm2[:])
            epsb = spool.tile([D, 1], f32, tag="epsb")
            nc.vector.memset(epsb[:], EPS)
            sd = scrpool.tile([D, C], f32, tag="scr")
            nc.scalar.activation(sd[:], var[:], Act.Sqrt, bias=epsb[:])
            inv = spool.tile([D, C], f32, tag="inv")
            nc.vector.reciprocal(inv[:], sd[:])

            a_t = spool.tile([D, C], f32, tag="a_t")
            nc.vector.tensor_mul(a_t[:], inv[:], gam[:])
            b_t = spool.tile([D, C], f32, tag="b_t")
            nc.vector.scalar_tensor_tensor(b_t[:], mean[:], -1.0, a_t[:],
                                           Alu.mult, Alu.mult)   # -mean*A
            nc.vector.tensor_add(b_t[:], bet[:], b_t[:])

            nc.vector.tensor_copy(AB2[0:D, 0:C], a_t[:])
            nc.vector.tensor_copy(AB2[0:D, C:2 * C], b_t[:])

            # ---- phase 2: gather A/B rows, FMA in bf16, cast-store ----
            ohTv = onehotT[:].rearrange("k (p i) -> k i p", i=CHUNKS)
            with tc.tile_pool(name="ps2", bufs=2, space="PSUM") as ps2:
                for g in range(GROUPS):
                    yb = ypool.tile([P, 8 * C], bf16)
                    for s in range(4 * g, 4 * g + 4):
                        q = s % 4
                        pab = ps2.tile([P, 4 * C], f32)
                        for k in range(2):
                            lt = ohTv[:, 2 * s + k, :]
                            nc.tensor.matmul(pab[:, k * C:(k + 1) * C],
                                             lt, AB2[:, 0:C],
                                             start=True, stop=True)
                            nc.tensor.matmul(pab[:, (2 + k) * C:(3 + k) * C],
                                             lt, AB2[:, C:2 * C],
                                             start=True, stop=True)
                        ab = abpool.tile([P, 4 * C], bf16)
                        if s % 8 == 0:
                            nc.vector.tensor_copy(ab[:], pab[:])
                        else:
                            nc.scalar.activation(ab[:], pab[:], Act.Copy)
                        ysl = yb[:, q * 2 * C:(q + 1) * 2 * C]
                        nc.vector.tensor_mul(
                            ysl, xs[g][:, q * 2 * C:(q + 1) * 2 * C],
                            ab[:, 0:2 * C])
                        nc.vector.tensor_add(ysl, ysl, ab[:, 2 * C:4 * C])
                    nc.gpsimd.dma_start(
                        Yv[:, 8 * g:8 * g + 8, :],
                        yb[:].rearrange("p (n c) -> p n c", c=C))

    nc.compile()
    return nc


def _get_program():
    if "nc" not in _CACHE:
        _CACHE["nc"] = _build_program()
    return _CACHE["nc"]


def kernel(X, d, parameter_t, fm_mean, gamma, beta):
    from concourse.bass_utils import run_bass_kernel_spmd

    X = np.ascontiguousarray(np.asarray(X), dtype=np.float32)
    d = np.ascontiguousarray(np.asarray(d), dtype=np.int32)
    gamma = np.ascontiguousarray(np.asarray(gamma), dtype=np.float32)
    beta = np.ascontiguousarray(np.asarray(beta), dtype=np.float32)

    nc = _get_program()
    in_maps = [
        {
            "X": X[c * SHARD:(c + 1) * SHARD],
            "d": d[c * SHARD:(c + 1) * SHARD],
            "gamma": gamma,
            "beta": beta,
        }
        for c in range(NCORES)
    ]
    res = run_bass_kernel_spmd(nc, in_maps, core_ids=list(range(NCORES)))
    out = np.concatenate([res.results[c]["Y"] for c in range(NCORES)], axis=0)
    return out.astype(np.float32, copy=False)
